# revision 1
# baseline (speedup 1.0000x reference)
"""Causal multi-head attention (B=64, T=256, C=384, H=6, D=64) on 8 TRN2 cores.

Strategy: data-parallel over batch (8 batches/core). Per (batch, head) the
attention is computed transposed -- S^T = K Q^T in [s, t] layout -- so softmax
row-sums come free from an interleaved [V|ones] matmul (Y^T rows + replicated
row-sum rows in one PSUM tile) and no PE transposes are needed anywhere.
QKV/score/projection matmuls run fp32r (fp32 rounded to 11-bit mantissa;
full PE rate at free-dim >= 256); the attention-weight (P) path runs bf16.

Algebraic folds (host-side):
  - K-bias and the q.b_k term cancel in row-softmax -> only Q carries bias,
    and the 1/sqrt(D) scale is folded into W_q and b_q.
  - V-bias passes through attention (softmax rows sum to 1) ->
    b_eff = b_proj + b_v @ W_proj, added during the projection evacuation.
"""
import sys

for _p in ("/opt/trn_rl_repo", "/root/.axon_site/_ro/trn_rl_repo"):
    if _p not in sys.path:
        sys.path.insert(0, _p)

import numpy as np

N_CORES = 8
B, T, C = 64, 256, 384
H, D = 6, 64
BS = B // N_CORES  # batches per core

_compiled = None


def _round_fp32r(x: np.ndarray) -> np.ndarray:
    """Round-to-nearest-even fp32 -> fp32r (11-bit mantissa), matching HW."""
    u = np.ascontiguousarray(x, dtype=np.float32).view(np.uint32).astype(np.uint64)
    lsb = (u >> 12) & 1
    u2 = ((u + 0x7FF + lsb) & 0xFFFFF000).astype(np.uint32)
    return u2.view(np.float32)


def _build():
    import concourse.bass as bass
    import concourse.bacc as bacc
    import concourse.tile as tile
    from concourse import mybir

    F32 = mybir.dt.float32
    F32R = mybir.dt.float32r
    BF16 = mybir.dt.bfloat16
    AF = mybir.ActivationFunctionType

    nc = bacc.Bacc(None)

    xt = nc.dram_tensor("xt", [BS, C, T], F32R, kind="ExternalInput")
    wq = nc.dram_tensor("wq", [C, 3 * C], F32R, kind="ExternalInput")
    wp = nc.dram_tensor("wp", [C, C], F32R, kind="ExternalInput")
    bqs = nc.dram_tensor("bqs", [128, 3], F32, kind="ExternalInput")
    beff = nc.dram_tensor("beff", [128, C], F32, kind="ExternalInput")
    mk = nc.dram_tensor("mk", [128, 2 * T], BF16, kind="ExternalInput")
    ones_d = nc.dram_tensor("ones_d", [128, C], BF16, kind="ExternalInput")
    y = nc.dram_tensor("y", [BS, T, C], F32, kind="ExternalOutput")

    with tile.TileContext(nc) as tc:
        with (
            tc.tile_pool(name="consts", bufs=1) as consts,
            tc.tile_pool(name="vperm", bufs=1) as vperm,
            tc.tile_pool(name="xts", bufs=3) as p_xts,
            tc.tile_pool(name="qkt", bufs=15) as p_qkt,
            tc.tile_pool(name="ptmp", bufs=4) as p_ptmp,
            tc.tile_pool(name="pr", bufs=4) as p_pr,
            tc.tile_pool(name="rbt", bufs=4) as p_rbt,
            tc.tile_pool(name="yct", bufs=6) as p_yct,
            tc.tile_pool(name="ysb", bufs=3) as p_ysb,
            tc.tile_pool(name="ps_big", bufs=3, space="PSUM") as ps_big,
            tc.tile_pool(name="ps_vy", bufs=2, space="PSUM") as ps_vy,
            tc.tile_pool(name="ps_yt", bufs=3, space="PSUM") as ps_yt,
        ):
            # ---- constants ----
            # batch-0 x load + Q-weights first: they gate the first matmuls
            xts0 = p_xts.tile([128, 3 * T], F32R, tag="xts", name="xts0")
            nc.sync.dma_start(
                out=xts0, in_=xt[0].rearrange("(j p) t -> p j t", p=128))
            bqs_sb = consts.tile([128, 3], F32, tag="bqs")
            nc.sync.dma_start(out=bqs_sb, in_=bqs[:, :])
            wq_sb, wp_sb = [], []
            for i in range(3):
                t_ = consts.tile([128, 3 * C], F32R, tag=f"wq{i}")
                wq_sb.append(t_)
            for lo, hi in ((0, C), (C, 2 * C), (2 * C, 3 * C)):
                for i in range(3):
                    nc.sync.dma_start(
                        out=wq_sb[i][:, lo:hi],
                        in_=wq[i * 128:(i + 1) * 128, lo:hi],
                    )
            # later-needed consts go on the ACT HWDGE queue (parallel issue)
            mk_sb = consts.tile([128, 2 * T], BF16, tag="mk")
            nc.scalar.dma_start(out=mk_sb, in_=mk[:, :])
            vaug = [[None, None], [None, None], [None, None]]
            for par in range(3):
                for sc in range(2):
                    t_ = vperm.tile([128, 6 * 128], BF16, tag=f"vaug{par}{sc}")
                    vaug[par][sc] = t_
                    dst = bass.AP(
                        tensor=t_.tensor,
                        offset=t_[:, :].offset + 64,
                        ap=[t_[:, :].ap[0], [256, 3], [64, 2], [1, 64]],
                    )
                    nc.scalar.dma_start(out=dst, in_=ones_d[:, :])
            for i in range(3):
                t2 = consts.tile([128, C], F32R, tag=f"wp{i}")
                nc.scalar.dma_start(out=t2, in_=wp[i * 128:(i + 1) * 128, :])
                wp_sb.append(t2)
            beff_sb = consts.tile([128, C], F32, tag="beff")
            nc.scalar.dma_start(out=beff_sb, in_=beff[:, :])

            # ---- per-batch pipeline (software-pipelined emission) ----
            state = {}

            def phase_qkv(b):
                par = b % 3
                if b == 0:
                    xts = xts0
                else:
                    xts = p_xts.tile([128, 3 * T], F32R, tag="xts",
                                     name=f"xts{b}")
                    nc.sync.dma_start(
                        out=xts,
                        in_=xt[b].rearrange("(j p) t -> p j t", p=128),
                    )
                qk_ps = []
                for jj in range(3):
                    pq = ps_big.tile([128, 2 * T], F32, tag="big",
                                     name=f"pq{b}_{jj}")
                    qk_ps.append(pq)
                    for half in range(2):
                        j = 2 * jj + half
                        for i in range(3):
                            nc.tensor.matmul(
                                pq[:, half * T:(half + 1) * T],
                                wq_sb[i][:, j * 128:(j + 1) * 128],
                                xts[:, i * T:(i + 1) * T],
                                start=(i == 0),
                                stop=(i == 2),
                            )
                qt = []
                for j in range(3):
                    dst = p_qkt.tile([128, T], F32R, tag="qkt",
                                     name=f"qt{b}_{j}")
                    nc.scalar.activation(
                        out=dst,
                        in_=qk_ps[j // 2][:, (j % 2) * T:(j % 2 + 1) * T],
                        func=AF.Identity, bias=bqs_sb[:, j:j + 1], scale=1.0,
                    )
                    qt.append(dst)
                kt3 = p_qkt.tile([128, T], F32R, tag="qkt", name=f"kt3_{b}")
                nc.scalar.activation(out=kt3, in_=qk_ps[1][:, T:2 * T],
                                     func=AF.Copy)
                kt45 = p_qkt.tile([128, 2 * T], F32R, tag="qkt2",
                                  name=f"kt45_{b}")
                nc.scalar.activation(out=kt45, in_=qk_ps[2][:, :], func=AF.Copy)

                for sc in range(2):
                    pv = ps_vy.tile([128, C], F32, tag="vy",
                                    name=f"pv{b}_{sc}")
                    for i in range(3):
                        nc.tensor.matmul(
                            pv,
                            xts[:, i * T + sc * 128:i * T + (sc + 1) * 128],
                            wq_sb[i][:, 2 * C:3 * C],
                            start=(i == 0),
                            stop=(i == 2),
                        )
                    vt = vaug[par][sc]
                    dst = bass.AP(
                        tensor=vt.tensor, offset=vt[:, :].offset,
                        ap=[vt[:, :].ap[0], [256, 3], [192, 2], [1, 64]],
                    )
                    srcap = bass.AP(
                        tensor=pv.tensor, offset=pv[:, :].offset,
                        ap=[pv[:, :].ap[0], [128, 3], [64, 2], [1, 64]],
                    )
                    nc.scalar.activation(out=dst, in_=srcap, func=AF.Copy)
                state[b] = (qt, kt3, kt45)

            def phase_heads(b):
                par = b % 3
                qt, kt3, kt45 = state[b]

                def kh_ap(h):
                    rb_ = (h % 2) * 64
                    hw = h // 2
                    if hw == 0:
                        return kt3[rb_:rb_ + 64, :]
                    return kt45[rb_:rb_ + 64, (hw - 1) * T:hw * T]

                yct = [
                    p_yct.tile([128, T], F32R, tag="yct", name=f"yct{b}_{j}")
                    for j in range(3)
                ]
                for h in range(6):
                    rbase = (h % 2) * 64
                    qh = qt[h // 2][rbase:rbase + 64, :]
                    kh = kh_ap(h)

                    pst = ps_big.tile([128, 2 * T], F32, tag="big",
                                      name=f"pst{b}_{h}")
                    for sc in range(2):
                        nc.tensor.matmul(
                            pst[:, sc * T:(sc + 1) * T],
                            kh[:, sc * 128:(sc + 1) * 128],
                            qh,
                            start=True,
                            stop=True,
                        )
                    ptmp = p_ptmp.tile([128, 2 * T], BF16, tag="ptmp",
                                       name=f"ptmp{b}_{h}")
                    nc.scalar.activation(out=ptmp, in_=pst, func=AF.Exp)
                    pr = p_pr.tile([128, 2 * T], BF16, tag="pr",
                                   name=f"pr{b}_{h}")
                    nc.vector.tensor_mul(pr, ptmp, mk_sb)

                    pyt = ps_yt.tile([128, T], F32, tag="yt",
                                     name=f"pyt{b}_{h}")
                    nc.tensor.matmul(
                        pyt,
                        vaug[par][0][:, h * 128:(h + 1) * 128],
                        pr[:, 0:T],
                        start=True,
                        stop=True,
                    )
                    nc.tensor.matmul(
                        pyt[:, 128:T],
                        vaug[par][1][:, h * 128:(h + 1) * 128],
                        pr[:, T + 128:2 * T],
                        start=False,
                        stop=True,
                    )
                    ybase, sbase = (0, 64) if h % 2 == 0 else (64, 0)
                    rbt = p_rbt.tile([128, T], F32R, tag="rbt",
                                     name=f"rbt{b}_{h}")
                    with nc.allow_low_precision(reason="softmax reciprocal"):
                        nc.vector.reciprocal(
                            out=rbt[rbase:rbase + 64, :],
                            in_=pyt[sbase:sbase + 64, :],
                        )
                    nc.vector.tensor_mul(
                        yct[h // 2][rbase:rbase + 64, :],
                        pyt[ybase:ybase + 64, :],
                        rbt[rbase:rbase + 64, :],
                    )
                state[b] = yct

            def phase_proj(b):
                yct = state.pop(b)
                ysb = p_ysb.tile([128, 2 * C], F32, tag="ysb",
                                 name=f"ysb{b}")
                for tck in range(2):
                    py = ps_vy.tile([128, C], F32, tag="vy",
                                    name=f"py{b}_{tck}")
                    for j in range(3):
                        nc.tensor.matmul(
                            py,
                            yct[j][:, tck * 128:(tck + 1) * 128],
                            wp_sb[j][:, :],
                            start=(j == 0),
                            stop=(j == 2),
                        )
                    nc.vector.tensor_add(
                        ysb[:, tck * C:(tck + 1) * C], py, beff_sb)
                nc.sync.dma_start(
                    out=y[b].rearrange("(tc p) c -> p tc c", p=128),
                    in_=ysb,
                )

            phase_qkv(0)
            phase_qkv(1)
            for b in range(BS):
                if b + 2 < BS:
                    phase_qkv(b + 2)
                phase_heads(b)
                phase_proj(b)

    nc.compile()
    return nc


def _get_compiled():
    global _compiled
    if _compiled is None:
        _compiled = _build()
    return _compiled


def _make_in_maps(x, W_qkv, b_qkv, W_proj, b_proj):
    x = np.asarray(x, dtype=np.float32)
    W_qkv = np.asarray(W_qkv, dtype=np.float32)
    b_qkv = np.asarray(b_qkv, dtype=np.float32)
    W_proj = np.asarray(W_proj, dtype=np.float32)
    b_proj = np.asarray(b_proj, dtype=np.float32)

    wq_mod = W_qkv.copy()
    wq_mod[:, :C] *= 0.125                      # fold attn scale into W_q
    wq_r = _round_fp32r(wq_mod)
    wp_r = _round_fp32r(W_proj)
    bqs = np.ascontiguousarray(
        (0.125 * b_qkv[:C]).reshape(3, 128).T, dtype=np.float32
    )
    beff = np.ascontiguousarray(
        np.broadcast_to(b_proj + b_qkv[2 * C:] @ W_proj, (128, C)),
        dtype=np.float32,
    )
    ti = np.arange(T)
    m0 = (ti[None, :] >= np.arange(128)[:, None]).astype(np.float32)
    m1 = (ti[None, :] >= (128 + np.arange(128))[:, None]).astype(np.float32)
    import ml_dtypes
    mk = np.ascontiguousarray(
        np.concatenate([m0, m1], axis=1)).astype(ml_dtypes.bfloat16)
    ones_d = np.ones((128, C), dtype=ml_dtypes.bfloat16)

    in_maps = []
    for c in range(N_CORES):
        xs = x[c * BS:(c + 1) * BS]                      # [BS, T, C]
        xtr = _round_fp32r(np.ascontiguousarray(xs.transpose(0, 2, 1)))
        in_maps.append({
            "xt": xtr, "wq": wq_r, "wp": wp_r, "bqs": bqs,
            "beff": beff, "mk": mk, "ones_d": ones_d,
        })
    return in_maps


def kernel(x, W_qkv, b_qkv, W_proj, b_proj):
    nc = _get_compiled()
    from concourse.bass_utils import run_bass_kernel_spmd

    in_maps = _make_in_maps(x, W_qkv, b_qkv, W_proj, b_proj)
    res = run_bass_kernel_spmd(nc, in_maps, core_ids=list(range(N_CORES)))
    out = np.concatenate([res.results[c]["y"] for c in range(N_CORES)], axis=0)
    return out.astype(np.float32)



# revision 6
# speedup vs baseline: 1.1940x; 1.1940x over previous
"""Causal multi-head attention (B=64, T=256, C=384, H=6, D=64) on 8 TRN2 cores.

Strategy: data-parallel over batch (8 batches/core). Per (batch, head) the
attention is computed transposed -- S^T = K Q^T in [s, t] layout -- so softmax
row-sums come free from an interleaved [V|ones] matmul (Y^T rows + replicated
row-sum rows in one PSUM tile) and no PE transposes are needed anywhere.

Work is spread over all four compute engines:
  - PE: QKV gen (f32r), scores (bf16, causally trimmed: s-chunk1 only needs
    t>=128), Y^T (bf16), projection (f32r).
  - Act: exp (PSUM->SBUF bf16, [128,384] trimmed), Q evac (+bias), V evac.
  - DVE: K evac, pair-fused reciprocal over a 2-head PSUM tile, scale-muls.
  - Pool (gpsimd): causal masking via in-place affine_select on the two
    128x128 diagonal blocks of each head's P tile (replaces a DVE mask-mul).

Algebraic folds (host side):
  - K-bias and the q.b_k term cancel in row-softmax -> only Q carries bias,
    and the 1/sqrt(D) scale is folded into W_q and b_q.
  - V-bias passes through attention (softmax rows sum to 1) and b_proj is
    affine -> both are added in a host epilogue (y += beff).
"""
import sys

for _p in ("/opt/trn_rl_repo", "/root/.axon_site/_ro/trn_rl_repo"):
    if _p not in sys.path:
        sys.path.insert(0, _p)

import numpy as np

N_CORES = 8
B, T, C = 64, 256, 384
H, D = 6, 64
BS = B // N_CORES  # batches per core

_compiled = None


def _round_fp32r(x: np.ndarray) -> np.ndarray:
    """Round-to-nearest-even fp32 -> fp32r (11-bit mantissa), matching HW."""
    u = np.ascontiguousarray(x, dtype=np.float32).view(np.uint32).astype(np.uint64)
    lsb = (u >> 12) & 1
    u2 = ((u + 0x7FF + lsb) & 0xFFFFF000).astype(np.uint32)
    return u2.view(np.float32)


def _build():
    import concourse.bass as bass
    import concourse.bacc as bacc
    import concourse.tile as tile
    from concourse import mybir

    F32 = mybir.dt.float32
    F32R = mybir.dt.float32r
    BF16 = mybir.dt.bfloat16
    AF = mybir.ActivationFunctionType

    nc = bacc.Bacc(None)

    xt = nc.dram_tensor("xt", [BS, C, T], F32R, kind="ExternalInput")
    wq = nc.dram_tensor("wq", [C, 3 * C], F32R, kind="ExternalInput")
    wp = nc.dram_tensor("wp", [C, C], F32R, kind="ExternalInput")
    bqs = nc.dram_tensor("bqs", [128, 3], F32, kind="ExternalInput")
    ones_d = nc.dram_tensor("ones_d", [128, C], BF16, kind="ExternalInput")
    y = nc.dram_tensor("y", [BS, T, C], F32, kind="ExternalOutput")

    with tile.TileContext(nc) as tc:
        with (
            tc.tile_pool(name="consts", bufs=1) as consts,
            tc.tile_pool(name="vperm", bufs=1) as vperm,
            tc.tile_pool(name="xts", bufs=4) as p_xts,
            tc.tile_pool(name="qkt", bufs=6) as p_qkt,
            tc.tile_pool(name="pr", bufs=10) as p_pr,
            tc.tile_pool(name="rbt", bufs=5) as p_rbt,
            tc.tile_pool(name="yct", bufs=9) as p_yct,
            tc.tile_pool(name="ysb", bufs=4) as p_ysb,
            # PSUM (8 banks x 2KB): qk-gen + proj-out share one [128,512]
            # pool (2 banks); scores + v-gen share a [128,384] pool (3
            # banks); Y/rowsum pair tiles get 3 banks.
            tc.tile_pool(name="ps_qk", bufs=2, space="PSUM") as ps_qk,
            tc.tile_pool(name="ps_st", bufs=3, space="PSUM") as ps_st,
            tc.tile_pool(name="ps_yt", bufs=3, space="PSUM") as ps_yt,
        ):
            # ---- constants ----
            # batch-0 x load + Q-weights first: they gate the first matmuls
            xts0 = p_xts.tile([128, 3 * T], F32R, tag="xts", name="xts0")
            nc.sync.dma_start(
                out=xts0, in_=xt[0].rearrange("(j p) t -> p j t", p=128))
            bqs_sb = consts.tile([128, 3], F32, tag="bqs")
            nc.sync.dma_start(out=bqs_sb, in_=bqs[:, :])
            wq_sb, wp_sb = [], []
            for i in range(3):
                t_ = consts.tile([128, 3 * C], F32R, tag=f"wq{i}")
                wq_sb.append(t_)
            for lo, hi in ((0, C), (C, 2 * C), (2 * C, 3 * C)):
                for i in range(3):
                    nc.sync.dma_start(
                        out=wq_sb[i][:, lo:hi],
                        in_=wq[i * 128:(i + 1) * 128, lo:hi],
                    )
            # later-needed consts go on the ACT HWDGE queue (parallel issue)
            vaug = [[None, None], [None, None], [None, None]]
            for par in range(3):
                for sc in range(2):
                    t_ = vperm.tile([128, 6 * 128], BF16, tag=f"vaug{par}{sc}")
                    vaug[par][sc] = t_
                    dst = bass.AP(
                        tensor=t_.tensor,
                        offset=t_[:, :].offset + 64,
                        ap=[t_[:, :].ap[0], [256, 3], [64, 2], [1, 64]],
                    )
                    nc.scalar.dma_start(out=dst, in_=ones_d[:, :])
            for i in range(3):
                t2 = consts.tile([128, C], F32R, tag=f"wp{i}")
                nc.scalar.dma_start(out=t2, in_=wp[i * 128:(i + 1) * 128, :])
                wp_sb.append(t2)

            fill0 = nc.gpsimd.to_reg(0.0)

            # ---- per-batch pipeline (software-pipelined emission) ----
            state = {}

            def phase_qkv(b):
                par = b % 3
                if b == 0:
                    xts = xts0
                else:
                    xts = p_xts.tile([128, 3 * T], F32R, tag="xts",
                                     name=f"xts{b}")
                    nc.sync.dma_start(
                        out=xts,
                        in_=xt[b].rearrange("(j p) t -> p j t", p=128),
                    )
                qk_ps = []
                for jj in range(3):
                    pq = ps_qk.tile([128, 2 * T], F32, tag="qk",
                                    name=f"pq{b}_{jj}")
                    qk_ps.append(pq)
                    for half in range(2):
                        j = 2 * jj + half
                        for i in range(3):
                            nc.tensor.matmul(
                                pq[:, half * T:(half + 1) * T],
                                wq_sb[i][:, j * 128:(j + 1) * 128],
                                xts[:, i * T:(i + 1) * T],
                                start=(i == 0),
                                stop=(i == 2),
                            )
                # Q evac (+per-chunk bias) on Act; K evac on DVE. Both bf16.
                qt = p_qkt.tile([128, 3 * T], BF16, tag="qkt", name=f"qt{b}")
                kt = p_qkt.tile([128, 3 * T], BF16, tag="qkt", name=f"kt{b}")
                for j in range(3):
                    nc.scalar.activation(
                        out=qt[:, j * T:(j + 1) * T],
                        in_=qk_ps[j // 2][:, (j % 2) * T:(j % 2 + 1) * T],
                        func=AF.Identity, bias=bqs_sb[:, j:j + 1], scale=1.0,
                    )
                nc.vector.tensor_copy(kt[:, 0:T], qk_ps[1][:, T:2 * T])
                nc.vector.tensor_copy(kt[:, T:3 * T], qk_ps[2][:, :])

                for sc in range(2):
                    pv = ps_st.tile([128, C], F32, tag="st",
                                     name=f"pv{b}_{sc}")
                    for i in range(3):
                        nc.tensor.matmul(
                            pv,
                            xts[:, i * T + sc * 128:i * T + (sc + 1) * 128],
                            wq_sb[i][:, 2 * C:3 * C],
                            start=(i == 0),
                            stop=(i == 2),
                        )
                    vt = vaug[par][sc]
                    dst = bass.AP(
                        tensor=vt.tensor, offset=vt[:, :].offset,
                        ap=[vt[:, :].ap[0], [256, 3], [192, 2], [1, 64]],
                    )
                    srcap = bass.AP(
                        tensor=pv.tensor, offset=pv[:, :].offset,
                        ap=[pv[:, :].ap[0], [128, 3], [64, 2], [1, 64]],
                    )
                    nc.scalar.activation(out=dst, in_=srcap, func=AF.Copy)
                state[b] = (qt, kt)

            def phase_heads(b):
                par = b % 3
                qt, kt = state[b]

                yct = [
                    p_yct.tile([128, T], F32R, tag="yct", name=f"yct{b}_{j}")
                    for j in range(3)
                ]
                for hp in range(3):
                    pyt = ps_yt.tile([128, 2 * T], F32, tag="yt",
                                     name=f"pyt{b}_{hp}")
                    for hh in range(2):
                        h = 2 * hp + hh
                        rb = hh * 64
                        qh = qt[rb:rb + 64, hp * T:(hp + 1) * T]
                        kh = kt[rb:rb + 64, hp * T:(hp + 1) * T]

                        pst = ps_st.tile([128, T + 128], F32, tag="st",
                                         name=f"pst{b}_{h}")
                        # S^T chunk0: s 0:128, all t. chunk1: s 128:256,
                        # causal -> only t 128:256 (written at cols 256:384).
                        nc.tensor.matmul(
                            pst[:, 0:T], kh[:, 0:128], qh,
                            start=True, stop=True,
                        )
                        nc.tensor.matmul(
                            pst[:, T:T + 128], kh[:, 128:T], qh[:, 128:T],
                            start=True, stop=True,
                        )
                        pr = p_pr.tile([128, T + 128], BF16, tag="pr",
                                       name=f"pr{b}_{h}")
                        nc.scalar.activation(out=pr, in_=pst[:, 0:T + 128],
                                             func=AF.Exp)
                        # causal zero-fill on the two diagonal 128x128 blocks
                        for lo in (0, T):
                            nc.gpsimd.affine_select(
                                out=pr[:, lo:lo + 128],
                                in_=pr[:, lo:lo + 128],
                                compare_op=mybir.AluOpType.is_ge,
                                fill=fill0,
                                base=0,
                                pattern=[[1, 128]],
                                channel_multiplier=-1,
                            )
                        off = hh * T
                        nc.tensor.matmul(
                            pyt[:, off:off + T],
                            vaug[par][0][:, h * 128:(h + 1) * 128],
                            pr[:, 0:T],
                            start=True,
                            stop=True,
                        )
                        nc.tensor.matmul(
                            pyt[:, off + 128:off + T],
                            vaug[par][1][:, h * 128:(h + 1) * 128],
                            pr[:, T:T + 128],
                            start=False,
                            stop=True,
                        )
                    # one reciprocal for the head pair (sums live at rows
                    # 64:128 for even head cols 0:256, rows 0:64 for odd head
                    # cols 256:512; recip of the Y halves is garbage, unread)
                    rbt = p_rbt.tile([128, 2 * T], F32R, tag="rbt",
                                     name=f"rbt{b}_{hp}")
                    with nc.allow_low_precision(reason="softmax reciprocal"):
                        nc.vector.reciprocal(out=rbt, in_=pyt)
                    nc.vector.tensor_mul(
                        yct[hp][0:64, :], pyt[0:64, 0:T], rbt[64:128, 0:T],
                    )
                    nc.vector.tensor_mul(
                        yct[hp][64:128, :], pyt[64:128, T:2 * T],
                        rbt[0:64, T:2 * T],
                    )
                state[b] = yct

            def phase_proj(b):
                yct = state.pop(b)
                ysb = p_ysb.tile([128, 2 * C], F32, tag="ysb",
                                 name=f"ysb{b}")
                for tck in range(2):
                    py = ps_qk.tile([128, C], F32, tag="qk",
                                    name=f"py{b}_{tck}")
                    for j in range(3):
                        nc.tensor.matmul(
                            py,
                            yct[j][:, tck * 128:(tck + 1) * 128],
                            wp_sb[j][:, :],
                            start=(j == 0),
                            stop=(j == 2),
                        )
                    if tck == 0:
                        nc.scalar.activation(
                            out=ysb[:, 0:C], in_=py, func=AF.Copy)
                    else:
                        nc.vector.tensor_copy(ysb[:, C:2 * C], py)
                nc.sync.dma_start(
                    out=y[b].rearrange("(tc p) c -> p tc c", p=128),
                    in_=ysb,
                )

            phase_qkv(0)
            phase_qkv(1)
            for b in range(BS):
                if b + 2 < BS:
                    phase_qkv(b + 2)
                phase_heads(b)
                phase_proj(b)

    nc.compile()
    return nc


def _get_compiled():
    global _compiled
    if _compiled is None:
        _compiled = _build()
    return _compiled


def _make_in_maps(x, W_qkv, b_qkv, W_proj, b_proj):
    x = np.asarray(x, dtype=np.float32)
    W_qkv = np.asarray(W_qkv, dtype=np.float32)
    b_qkv = np.asarray(b_qkv, dtype=np.float32)
    W_proj = np.asarray(W_proj, dtype=np.float32)

    wq_mod = W_qkv.copy()
    wq_mod[:, :C] *= 0.125                      # fold attn scale into W_q
    wq_r = _round_fp32r(wq_mod)
    wp_r = _round_fp32r(W_proj)
    bqs = np.ascontiguousarray(
        (0.125 * b_qkv[:C]).reshape(3, 128).T, dtype=np.float32
    )
    import ml_dtypes
    ones_d = np.ones((128, C), dtype=ml_dtypes.bfloat16)

    in_maps = []
    for c in range(N_CORES):
        xs = x[c * BS:(c + 1) * BS]                      # [BS, T, C]
        xtr = _round_fp32r(np.ascontiguousarray(xs.transpose(0, 2, 1)))
        in_maps.append({
            "xt": xtr, "wq": wq_r, "wp": wp_r, "bqs": bqs,
            "ones_d": ones_d,
        })
    return in_maps


def kernel(x, W_qkv, b_qkv, W_proj, b_proj):
    nc = _get_compiled()
    from concourse.bass_utils import run_bass_kernel_spmd

    in_maps = _make_in_maps(x, W_qkv, b_qkv, W_proj, b_proj)
    res = run_bass_kernel_spmd(nc, in_maps, core_ids=list(range(N_CORES)))
    out = np.concatenate([res.results[c]["y"] for c in range(N_CORES)], axis=0)
    # V-bias passes through attention; b_proj is affine: host epilogue.
    beff = (np.asarray(b_proj, dtype=np.float32)
            + np.asarray(b_qkv, dtype=np.float32)[2 * C:]
            @ np.asarray(W_proj, dtype=np.float32))
    return (out + beff).astype(np.float32)


# revision 22
# speedup vs baseline: 1.2405x; 1.0389x over previous
"""Causal multi-head attention (B=64, T=256, C=384, H=6, D=64) on 8 TRN2 cores.

Strategy: data-parallel over batch (8 batches/core). Per (batch, head) the
attention is computed transposed -- S^T = K Q^T in [s, t] layout -- so softmax
row-sums come free from an interleaved [V|ones] matmul (Y^T rows + replicated
row-sum rows in one PSUM tile) and no PE transposes are needed anywhere.

Work is spread over all four compute engines:
  - PE: QKV gen (f32r), scores (bf16, causally trimmed: s-chunk1 only needs
    t>=128), Y^T (bf16), projection (f32r).
  - Act: exp (PSUM->SBUF bf16, [128,384] trimmed), Q evac (+bias), V evac.
  - DVE: K evac, pair-fused reciprocal over a 2-head PSUM tile, scale-muls.
  - Pool (gpsimd): causal masking via in-place affine_select on the two
    128x128 diagonal blocks of each head's P tile (replaces a DVE mask-mul).

Algebraic folds (host side):
  - K-bias and the q.b_k term cancel in row-softmax -> only Q carries bias,
    and the 1/sqrt(D) scale is folded into W_q and b_q.
  - V-bias passes through attention (softmax rows sum to 1) and b_proj is
    affine -> both are added in a host epilogue (y += beff).
"""
import sys

for _p in ("/opt/trn_rl_repo", "/root/.axon_site/_ro/trn_rl_repo"):
    if _p not in sys.path:
        sys.path.insert(0, _p)

import numpy as np

N_CORES = 8
B, T, C = 64, 256, 384
H, D = 6, 64
BS = B // N_CORES  # batches per core

_compiled = None


def _round_fp32r(x: np.ndarray) -> np.ndarray:
    """Round-to-nearest-even fp32 -> fp32r (11-bit mantissa), matching HW."""
    u = np.ascontiguousarray(x, dtype=np.float32).view(np.uint32).astype(np.uint64)
    lsb = (u >> 12) & 1
    u2 = ((u + 0x7FF + lsb) & 0xFFFFF000).astype(np.uint32)
    return u2.view(np.float32)


def _build():
    import concourse.bass as bass
    import concourse.bacc as bacc
    import concourse.tile as tile
    from concourse import mybir

    F32 = mybir.dt.float32
    F32R = mybir.dt.float32r
    BF16 = mybir.dt.bfloat16
    AF = mybir.ActivationFunctionType

    nc = bacc.Bacc(None)

    xt = nc.dram_tensor("xt", [BS, C, T], F32R, kind="ExternalInput")
    wq = nc.dram_tensor("wq", [C, 3 * C], F32R, kind="ExternalInput")
    wp = nc.dram_tensor("wp", [C, C], F32R, kind="ExternalInput")
    bqs = nc.dram_tensor("bqs", [128, 3], F32, kind="ExternalInput")
    ones_d = nc.dram_tensor("ones_d", [128, 2 * C], BF16, kind="ExternalInput")
    y = nc.dram_tensor("y", [BS, T, C], F32, kind="ExternalOutput")

    with tile.TileContext(nc) as tc:
        with (
            tc.tile_pool(name="consts", bufs=1) as consts,
            tc.tile_pool(name="vperm", bufs=1) as vperm,
            tc.tile_pool(name="xts", bufs=4) as p_xts,
            tc.tile_pool(name="qkt", bufs=6) as p_qkt,
            tc.tile_pool(name="pr", bufs=10) as p_pr,
            tc.tile_pool(name="rbt", bufs=5) as p_rbt,
            tc.tile_pool(name="yct", bufs=9) as p_yct,
            tc.tile_pool(name="ysb", bufs=4) as p_ysb,
            # PSUM (8 banks x 2KB): qk-gen + proj-out share one [128,512]
            # pool (2 banks); scores + v-gen share a [128,384] pool (3
            # banks); Y/rowsum pair tiles get 3 banks.
            tc.tile_pool(name="ps_qk", bufs=2, space="PSUM") as ps_qk,
            tc.tile_pool(name="ps_st", bufs=3, space="PSUM") as ps_st,
            tc.tile_pool(name="ps_yt", bufs=3, space="PSUM") as ps_yt,
        ):
            # ---- constants ----
            # batch-0 x load + Q-weights first: they gate the first matmuls
            xts0 = p_xts.tile([128, 3 * T], F32R, tag="xts", name="xts0")
            nc.sync.dma_start(
                out=xts0, in_=xt[0].rearrange("(j p) t -> p j t", p=128))
            wq_sb, wp_sb = [], []
            for i in range(3):
                t_ = consts.tile([128, 3 * C], F32R, tag=f"wq{i}")
                wq_sb.append(t_)
            # startup load order: Q weights striped per contraction chunk on
            # the sync ring (right behind x0) so QK matmuls start ASAP; K and
            # V weights land in parallel on the scalar ring, K first.
            for i in range(3):
                nc.sync.dma_start(
                    out=wq_sb[i][:, 0:C], in_=wq[i * 128:(i + 1) * 128, 0:C])
            for lo, hi in ((C, 2 * C), (2 * C, 3 * C)):
                for i in range(3):
                    nc.scalar.dma_start(
                        out=wq_sb[i][:, lo:hi],
                        in_=wq[i * 128:(i + 1) * 128, lo:hi],
                    )
            bqs_sb = consts.tile([128, 3], F32, tag="bqs")
            nc.sync.dma_start(out=bqs_sb, in_=bqs[:, :])
            # later-needed consts go on the ACT HWDGE queue (parallel issue)
            vaug = []
            for par in range(3):
                t_ = vperm.tile([128, 2 * 6 * 128], BF16, tag=f"vaug{par}")
                vaug.append(t_)
                dst = bass.AP(
                    tensor=t_.tensor,
                    offset=t_[:, :].offset + 64,
                    ap=[t_[:, :].ap[0], [256, 6], [64, 2], [1, 64]],
                )
                nc.scalar.dma_start(out=dst, in_=ones_d[:, :])
            wp_t = consts.tile([128, 3 * C], F32R, tag="wp")
            nc.scalar.dma_start(
                out=wp_t, in_=wp.rearrange("(j p) c -> p j c", p=128))
            for i in range(3):
                wp_sb.append(wp_t[:, i * C:(i + 1) * C])

            fill0 = nc.gpsimd.to_reg(0.0)

            # ---- per-batch pipeline (software-pipelined emission) ----
            state = {}

            xts_tiles = {0: xts0}

            def phase_xdma(b):
                xts = p_xts.tile([128, 3 * T], F32R, tag="xts",
                                 name=f"xts{b}")
                nc.sync.dma_start(
                    out=xts,
                    in_=xt[b].rearrange("(j p) t -> p j t", p=128),
                )
                xts_tiles[b] = xts

            def phase_qkv(b):
                par = b % 3
                xts = xts_tiles.pop(b)
                qk_ps = []
                for jj in range(3):
                    pq = ps_qk.tile([128, 2 * T], F32, tag="qk",
                                    name=f"pq{b}_{jj}")
                    qk_ps.append(pq)
                    for half in range(2):
                        j = 2 * jj + half
                        for i in range(3):
                            nc.tensor.matmul(
                                pq[:, half * T:(half + 1) * T],
                                wq_sb[i][:, j * 128:(j + 1) * 128],
                                xts[:, i * T:(i + 1) * T],
                                start=(i == 0),
                                stop=(i == 2),
                            )
                # Q evac (+per-chunk bias) on Act; K evac on DVE. Both bf16.
                qt = p_qkt.tile([128, 3 * T], BF16, tag="qkt", name=f"qt{b}")
                kt = p_qkt.tile([128, 3 * T], BF16, tag="qkt", name=f"kt{b}")
                for j in range(3):
                    nc.scalar.activation(
                        out=qt[:, j * T:(j + 1) * T],
                        in_=qk_ps[j // 2][:, (j % 2) * T:(j % 2 + 1) * T],
                        func=AF.Identity, bias=bqs_sb[:, j:j + 1], scale=1.0,
                    )
                nc.vector.tensor_copy(kt[:, 0:T], qk_ps[1][:, T:2 * T])
                nc.vector.tensor_copy(kt[:, T:3 * T], qk_ps[2][:, :])

                for sc in range(2):
                    pv = ps_st.tile([128, C], F32, tag="st",
                                     name=f"pv{b}_{sc}")
                    for i in range(3):
                        nc.tensor.matmul(
                            pv,
                            xts[:, i * T + sc * 128:i * T + (sc + 1) * 128],
                            wq_sb[i][:, 2 * C:3 * C],
                            start=(i == 0),
                            stop=(i == 2),
                        )
                    vt = vaug[par]
                    dst = bass.AP(
                        tensor=vt.tensor, offset=vt[:, :].offset + sc * 768,
                        ap=[vt[:, :].ap[0], [256, 3], [192, 2], [1, 64]],
                    )
                    srcap = bass.AP(
                        tensor=pv.tensor, offset=pv[:, :].offset,
                        ap=[pv[:, :].ap[0], [128, 3], [64, 2], [1, 64]],
                    )
                    nc.scalar.activation(out=dst, in_=srcap, func=AF.Copy)
                state[b] = (qt, kt)

            def phase_heads(b):
                par = b % 3
                qt, kt = state[b]

                yct = [
                    p_yct.tile([128, T], F32R, tag="yct", name=f"yct{b}_{j}")
                    for j in range(3)
                ]
                for hp in range(3):
                    pyt = ps_yt.tile([128, 2 * T], F32, tag="yt",
                                     name=f"pyt{b}_{hp}")
                    for hh in range(2):
                        h = 2 * hp + hh
                        rb = hh * 64
                        qh = qt[rb:rb + 64, hp * T:(hp + 1) * T]
                        kh = kt[rb:rb + 64, hp * T:(hp + 1) * T]

                        pst = ps_st.tile([128, T + 128], F32, tag="st",
                                         name=f"pst{b}_{h}")
                        # S^T chunk0: s 0:128, all t. chunk1: s 128:256,
                        # causal -> only t 128:256 (written at cols 256:384).
                        nc.tensor.matmul(
                            pst[:, 0:T], kh[:, 0:128], qh,
                            start=True, stop=True,
                        )
                        nc.tensor.matmul(
                            pst[:, T:T + 128], kh[:, 128:T], qh[:, 128:T],
                            start=True, stop=True,
                        )
                        pr = p_pr.tile([128, T + 128], BF16, tag="pr",
                                       name=f"pr{b}_{h}")
                        nc.scalar.activation(out=pr, in_=pst[:, 0:T + 128],
                                             func=AF.Exp)
                        # causal zero-fill on the two diagonal 128x128 blocks
                        for lo in (0, T):
                            nc.gpsimd.affine_select(
                                out=pr[:, lo:lo + 128],
                                in_=pr[:, lo:lo + 128],
                                compare_op=mybir.AluOpType.is_ge,
                                fill=fill0,
                                base=0,
                                pattern=[[1, 128]],
                                channel_multiplier=-1,
                            )
                        off = hh * T
                        nc.tensor.matmul(
                            pyt[:, off:off + T],
                            vaug[par][:, h * 128:(h + 1) * 128],
                            pr[:, 0:T],
                            start=True,
                            stop=True,
                        )
                        nc.tensor.matmul(
                            pyt[:, off + 128:off + T],
                            vaug[par][:, 768 + h * 128:768 + (h + 1) * 128],
                            pr[:, T:T + 128],
                            start=False,
                            stop=True,
                        )
                    # one reciprocal for the head pair (sums live at rows
                    # 64:128 for even head cols 0:256, rows 0:64 for odd head
                    # cols 256:512; recip of the Y halves is garbage, unread)
                    rbt = p_rbt.tile([128, 2 * T], F32R, tag="rbt",
                                     name=f"rbt{b}_{hp}")
                    with nc.allow_low_precision(reason="softmax reciprocal"):
                        nc.vector.reciprocal(out=rbt, in_=pyt)
                    nc.vector.tensor_mul(
                        yct[hp][0:64, :], pyt[0:64, 0:T], rbt[64:128, 0:T],
                    )
                    nc.vector.tensor_mul(
                        yct[hp][64:128, :], pyt[64:128, T:2 * T],
                        rbt[0:64, T:2 * T],
                    )
                state[b] = yct

            def phase_proj(b):
                yct = state.pop(b)
                ysb = p_ysb.tile([128, 2 * C], F32, tag="ysb",
                                 name=f"ysb{b}")
                for tck in range(2):
                    py = ps_qk.tile([128, C], F32, tag="qk",
                                    name=f"py{b}_{tck}")
                    for j in range(3):
                        nc.tensor.matmul(
                            py,
                            yct[j][:, tck * 128:(tck + 1) * 128],
                            wp_sb[j][:, :],
                            start=(j == 0),
                            stop=(j == 2),
                        )
                    if tck == 0:
                        nc.scalar.activation(
                            out=ysb[:, 0:C], in_=py, func=AF.Copy)
                    else:
                        nc.vector.tensor_copy(ysb[:, C:2 * C], py)
                nc.sync.dma_start(
                    out=y[b].rearrange("(tc p) c -> p tc c", p=128),
                    in_=ysb,
                )

            phase_xdma(1)
            phase_qkv(0)
            phase_xdma(2)
            phase_qkv(1)
            for b in range(BS):
                if b + 3 < BS:
                    phase_xdma(b + 3)
                phase_heads(b)
                if b + 2 < BS:
                    phase_qkv(b + 2)
                phase_proj(b)

    nc.compile()
    return nc


def _get_compiled():
    global _compiled
    if _compiled is None:
        _compiled = _build()
    return _compiled


def _make_in_maps(x, W_qkv, b_qkv, W_proj, b_proj):
    x = np.asarray(x, dtype=np.float32)
    W_qkv = np.asarray(W_qkv, dtype=np.float32)
    b_qkv = np.asarray(b_qkv, dtype=np.float32)
    W_proj = np.asarray(W_proj, dtype=np.float32)

    wq_mod = W_qkv.copy()
    wq_mod[:, :C] *= 0.125                      # fold attn scale into W_q
    wq_r = _round_fp32r(wq_mod)
    wp_r = _round_fp32r(W_proj)
    bqs = np.ascontiguousarray(
        (0.125 * b_qkv[:C]).reshape(3, 128).T, dtype=np.float32
    )
    import ml_dtypes
    ones_d = np.ones((128, 2 * C), dtype=ml_dtypes.bfloat16)

    in_maps = []
    for c in range(N_CORES):
        xs = x[c * BS:(c + 1) * BS]                      # [BS, T, C]
        xtr = _round_fp32r(np.ascontiguousarray(xs.transpose(0, 2, 1)))
        in_maps.append({
            "xt": xtr, "wq": wq_r, "wp": wp_r, "bqs": bqs,
            "ones_d": ones_d,
        })
    return in_maps


def kernel(x, W_qkv, b_qkv, W_proj, b_proj):
    nc = _get_compiled()
    from concourse.bass_utils import run_bass_kernel_spmd

    in_maps = _make_in_maps(x, W_qkv, b_qkv, W_proj, b_proj)
    res = run_bass_kernel_spmd(nc, in_maps, core_ids=list(range(N_CORES)))
    out = np.concatenate([res.results[c]["y"] for c in range(N_CORES)], axis=0)
    # V-bias passes through attention; b_proj is affine: host epilogue.
    beff = (np.asarray(b_proj, dtype=np.float32)
            + np.asarray(b_qkv, dtype=np.float32)[2 * C:]
            @ np.asarray(W_proj, dtype=np.float32))
    return (out + beff).astype(np.float32)
